# revision 29
# baseline (speedup 1.0000x reference)
"""HODLR matvec kernel for 8 TRN2 NeuronCores (Bass/Tile), v3.

Sharding: node axis split into 8 contiguous slices of 32768 nodes.
DMA-bound design (~40MB/core at ~360GB/s): compute hides under the
input stream.

Per core:
  stream order: xt | UPA (u levels 0-2, n-major) | UPB (levels 3-7) |
    UT23 (u^T levels 4-7, fully prefetched) | UT01 (levels 0-3, ring),
    with corr output DMAs at the queue tails.
  projection  one DR fp8 matmul per chunk-pair per level-group with x
    as the stationary; per-block PSUM accumulators (L7 ping-pongs two
    banks, coarser levels ride 1-bank rings; levels 0-2 close as soon
    as UPA lands, launching the AllGather early).
  collective  AllGather of the [64, 192] level-0..2 partials; masked
    receive-combine (sender-invariant).
  transposes  PE is_transpose matmuls turn sibling-selected t^T slots
    into fp8 DR stationaries (sibling XOR encoded in the source APs).
  expansion   per 512-node group: one DR matmul vs UT23 (levels 4-7,
    PSUM start) and one vs UT01 (levels 0-3, stop); drains write bf16
    corr staged per 4096 block.
u/x fed as fp8e4m3 (u scaled by USCALE; host divides the correction by
USCALE^2 and adds diag*x in fp32).
"""

import os
import sys

sys.path.insert(0, "/opt/trn_rl_repo")

import numpy as np
import ml_dtypes

BF16 = ml_dtypes.bfloat16
FP8 = ml_dtypes.float8_e4m3

B = 64
N = 262144
NCORES = 8
M = N // NCORES          # 32768 nodes per core
R = 64
DEPTH = 8
CH = M // 128            # 256 chunks of 128 nodes
CP = CH // 2             # 128 chunk-pairs (256 nodes, DR k-tiles)
USCALE = 64.0

_cached = {}


def _build_bass():
    import concourse.bacc as bacc
    import concourse.tile as tile
    import concourse.mybir as mybir
    from contextlib import ExitStack

    BF = mybir.dt.bfloat16
    F8 = mybir.dt.float8e4
    F32 = mybir.dt.float32
    ADD = mybir.AluOpType.add
    MULT = mybir.AluOpType.mult
    DR = mybir.MatmulPerfMode.DoubleRow

    nc = bacc.Bacc(
        "TRN2",
        target_bir_lowering=False,
        debug=False,
        enable_asserts=False,
        num_devices=NCORES,
    )

    xt_d = nc.dram_tensor("xt", [128, CH, B], F8, kind="ExternalInput").ap()
    upa_d = nc.dram_tensor("upa", [128, CP, 2, 192], F8, kind="ExternalInput").ap()
    upb_d = nc.dram_tensor("upb", [128, CP, 2, 320], F8, kind="ExternalInput").ap()
    ut_d = nc.dram_tensor("ut", [4, 128, M], F8, kind="ExternalInput").ap()
    msk_d = nc.dram_tensor("mask", [64, 8, 192], BF, kind="ExternalInput").ap()
    idn_d = nc.dram_tensor("idn", [64, 64], BF, kind="ExternalInput").ap()
    corr_d = nc.dram_tensor("corr", [B, M], F8, kind="ExternalOutput").ap()

    NSL = 8            # slices for UPA/UPB streams
    SLC = CP // NSL    # 16 chunk-pairs per slice
    # TT slot index layout (t^T vectors, [B, 64] each)
    SL7 = 0            # 32 slots: level-7 blocks
    SL6 = 32           # 16
    SL5 = 48           # 8
    SL4 = 56           # 4
    SL3 = 60           # 2
    SL0 = 62           # 3: combined levels 0,1,2 (post-collective)
    NSLOT = 65

    with tile.TileContext(nc) as tc, ExitStack() as ctx:
        const = ctx.enter_context(tc.tile_pool(name="const", bufs=1))
        upap = ctx.enter_context(tc.tile_pool(name="upap", bufs=3))
        upbp = ctx.enter_context(tc.tile_pool(name="upbp", bufs=3))
        utap = ctx.enter_context(tc.tile_pool(name="utap", bufs=8))
        utbp = ctx.enter_context(tc.tile_pool(name="utbp", bufs=4))
        ttp = ctx.enter_context(tc.tile_pool(name="ttp", bufs=1))
        statp = ctx.enter_context(tc.tile_pool(name="statp", bufs=1))
        yop = ctx.enter_context(tc.tile_pool(name="yop", bufs=3))
        # PSUM: 8 banks: prA bufs=2 (L7 ping-pong, then transposes),
        # prB..prF bufs=1 (L6..L3 + psF, then the expansion eps ring
        # together with prA).
        prA = ctx.enter_context(tc.tile_pool(name="prA", bufs=2, space="PSUM"))
        prB = ctx.enter_context(tc.tile_pool(name="prB", bufs=1, space="PSUM"))
        prC = ctx.enter_context(tc.tile_pool(name="prC", bufs=1, space="PSUM"))
        prD = ctx.enter_context(tc.tile_pool(name="prD", bufs=1, space="PSUM"))
        prE = ctx.enter_context(tc.tile_pool(name="prE", bufs=1, space="PSUM"))
        prF = ctx.enter_context(tc.tile_pool(name="prF", bufs=1, space="PSUM"))
        dram = ctx.enter_context(tc.tile_pool(name="dram", bufs=1, space="DRAM"))

        def drainer():
            # gpsimd cannot access PSUM; DVE mostly, scheduler-picked 1/3
            i = 0
            while True:
                yield (nc.vector if i % 3 != 2 else nc.any)
                i += 1

        dr_eng = drainer()

        # ---------------- constants ----------------
        xt = const.tile([128, CH, B], F8, tag="xt")
        nc.sync.dma_start(xt[:, 0 : CH // 2, :], xt_d[:, 0 : CH // 2, :])
        nc.scalar.dma_start(xt[:, CH // 2 :, :], xt_d[:, CH // 2 :, :])
        msk = const.tile([64, 8, 192], BF, tag="msk")
        nc.scalar.dma_start(msk[:], msk_d[:])
        idn = const.tile([64, 64], BF, tag="idn")
        nc.scalar.dma_start(idn[:], idn_d[:])

        TT = ttp.tile([64, NSLOT, 64], BF, tag="TT")

        # ---------------- projection: levels 0-2 (UPA) ----------------
        psF_t = prF.tile([64, 512], F32, tag="t", name="psF")
        psF = psF_t[:, 0:192]
        for s in range(NSL):
            ua = upap.tile([128, SLC, 2, 192], F8, tag="upa", name=f"upa{s}")
            (nc.sync if s % 2 == 0 else nc.scalar).dma_start(
                ua[:], upa_d[:, SLC * s : SLC * (s + 1), :, :]
            )
            for i in range(SLC):
                cp = SLC * s + i
                nc.tensor.matmul(
                    psF,
                    xt[:, 2 * cp : 2 * cp + 2, :],
                    ua[:, i, :, :],
                    start=(cp == 0),
                    stop=(cp == CP - 1),
                    perf_mode=DR,
                )

        # collective staging + AllGather (launches ~25% into the stream)
        b_in = dram.tile([64, 192], BF, tag="b_in")
        b_out = dram.tile([8, 64, 192], BF, tag="b_out", addr_space="Shared")
        sb_in = statp.tile([64, 192], BF, tag="sb_in")
        nc.vector.tensor_copy(sb_in[:], psF)
        nc.gpsimd.dma_start(b_in[:], sb_in[:])
        nc.gpsimd.collective_compute(
            "AllGather",
            mybir.AluOpType.bypass,
            replica_groups=[list(range(NCORES))],
            ins=[b_in.opt()],
            outs=[b_out.opt()],
        )
        recv = statp.tile([64, 8, 192], BF, tag="recv")
        for k in range(8):
            nc.gpsimd.dma_start(recv[:, k, :], b_out[k, :, :])
        # masked receive-combine -> TT slots 62..64 (levels 0,1,2)
        mtmp = statp.tile([64, 8, 192], BF, tag="mtmp")
        for k in range(8):
            nc.vector.tensor_tensor(
                mtmp[:, k, :], recv[:, k, :], msk[:, k, :], op=MULT
            )
        acc01 = statp.tile([64, 192], BF, tag="acc01")
        nc.vector.tensor_tensor(acc01[:], mtmp[:, 0, :], mtmp[:, 1, :], op=ADD)
        for k in range(2, 7):
            nc.vector.tensor_tensor(acc01[:], acc01[:], mtmp[:, k, :], op=ADD)
        nc.vector.tensor_tensor(
            TT[:, SL0 : SL0 + 3, :], acc01[:], mtmp[:, 7, :], op=ADD
        )

        # ---------------- projection: levels 3-7 (UPB) ----------------
        # per-level per-block psum accumulators; close cadences (cps):
        # L3:64 L4:32 L5:16 L6:8 L7:4
        lvl_pool = {3: prE, 4: prD, 5: prC, 6: prB, 7: prA}
        lvl_slot = {3: SL3, 4: SL4, 5: SL5, 6: SL6, 7: SL7}
        lvl_cad = {3: 64, 4: 32, 5: 16, 6: 8, 7: 4}
        cur = {}
        for s in range(NSL):
            ub = upbp.tile([128, SLC, 2, 320], F8, tag="upb", name=f"upb{s}")
            (nc.sync if s % 2 == 0 else nc.scalar).dma_start(
                ub[:], upb_d[:, SLC * s : SLC * (s + 1), :, :]
            )
            for i in range(SLC):
                cp = SLC * s + i
                for l in (7, 6, 5, 4, 3):
                    cad = lvl_cad[l]
                    blk = cp // cad
                    j = l - 3
                    if cp % cad == 0:
                        cur[l] = lvl_pool[l].tile(
                            [64, 512], F32, tag="t", name=f"ps{l}_{blk}"
                        )
                    nc.tensor.matmul(
                        cur[l][:, 0:64],
                        xt[:, 2 * cp : 2 * cp + 2, :],
                        ub[:, i, :, 64 * j : 64 * j + 64],
                        start=(cp % cad == 0),
                        stop=(cp % cad == cad - 1),
                        perf_mode=DR,
                    )
                    if cp % cad == cad - 1:
                        next(dr_eng).tensor_copy(
                            TT[:, lvl_slot[l] + blk, :], cur[l][:, 0:64]
                        )

        # ---------------- sibling transposes -> fp8 stationaries -------
        # P23[:, m7, 0, :] = (t4sib | t5sib); [:, m7, 1, :] = (t6sib | t7sib)
        # P01[:, m3, 0, :] = (L0comb | L1comb); [:, m3, 1, :] = (L2comb | t3sib)
        P23 = statp.tile([128, 32, 2, B], F8, tag="P23")
        P01 = statp.tile([128, 2, 2, B], F8, tag="P01")

        def transpose_pair(dst_ap, slot_top, slot_bot, nm):
            # full-bank scratch keeps start-flag zeroing away from
            # neighbors
            tp = prA.tile([128, 1024], BF, tag="t", name=nm)
            nc.tensor.matmul(
                tp[0:64, 0:B], TT[:, slot_top, :], idn[:],
                is_transpose=True,
            )
            nc.tensor.matmul(
                tp[64:128, 0:B], TT[:, slot_bot, :], idn[:],
                is_transpose=True,
            )
            next(dr_eng).tensor_copy(dst_ap, tp[:, 0:B])

        for m7 in range(32):
            transpose_pair(
                P23[:, m7, 0, :],
                SL4 + ((m7 // 8) ^ 1),
                SL5 + ((m7 // 4) ^ 1),
                f"tp45_{m7}",
            )
            transpose_pair(
                P23[:, m7, 1, :],
                SL6 + ((m7 // 2) ^ 1),
                SL7 + (m7 ^ 1),
                f"tp67_{m7}",
            )
        # P01 last: it waits on the collective, and the PE runs in order
        for m3 in range(2):
            transpose_pair(P01[:, m3, 0, :], SL0 + 0, SL0 + 1, f"tp01_{m3}")
            transpose_pair(
                P01[:, m3, 1, :], SL0 + 2, SL3 + (m3 ^ 1), f"tp23_{m3}"
            )

        # ---------------- UT prefetch ----------------
        # UT23 fully resident (no backpressure); UT01 4-deep ring
        u23s = []
        for gb in range(8):
            u23 = utap.tile([128, 2, 4096], F8, tag="ut23", name=f"u23_{gb}")
            for f in range(2):
                (nc.sync if gb % 2 == 0 else nc.scalar).dma_start(
                    u23[:, f, :], ut_d[2 + f, :, 4096 * gb : 4096 * (gb + 1)]
                )
            u23s.append(u23)

        # ---------------- expansion (per 4096-node block) ----------------
        erng = (prB, prC, prD, prE, prF, prA)
        for gb in range(8):
            u01 = utbp.tile([128, 2, 4096], F8, tag="ut01", name=f"u01_{gb}")
            for f in range(2):
                (nc.scalar if gb % 2 == 0 else nc.sync).dma_start(
                    u01[:, f, :], ut_d[f, :, 4096 * gb : 4096 * (gb + 1)]
                )
            yo = yop.tile([B, 4096], F8, tag="yo", name=f"yo{gb}")
            for gg in range(8):
                g = 8 * gb + gg
                eps_t = erng[g % 6].tile(
                    [64, 512], F32, tag="t", name=f"eps{g}"
                )
                eps = eps_t[:, 0:512]
                sl = slice(512 * gg, 512 * (gg + 1))
                nc.tensor.matmul(
                    eps, P23[:, g // 2, :, :], u23s[gb][:, :, sl],
                    start=True, stop=False, perf_mode=DR,
                )
                nc.tensor.matmul(
                    eps, P01[:, g // 32, :, :], u01[:, :, sl],
                    start=False, stop=True, perf_mode=DR,
                )
                next(dr_eng).tensor_copy(yo[:, sl], eps)
            (nc.sync if gb % 2 == 0 else nc.scalar).dma_start(
                corr_d[:, 4096 * gb : 4096 * (gb + 1)], yo[:]
            )

    nc.compile()
    return nc


def _pack_inputs(x, diag, u):
    """Build per-core input maps. x (B,N,1) f32, u (DEPTH,N,R) f32."""
    in_maps = []
    x2 = np.asarray(x).reshape(B, N)
    u3 = np.asarray(u)
    idn = np.eye(64, dtype=BF16)
    for c in range(NCORES):
        base = c * M
        xsl = x2[:, base : base + M]                      # (B, M)
        us = (u3[:, base : base + M, :] * USCALE).astype(np.float32)
        xt = np.ascontiguousarray(
            xsl.T.reshape(CH, 128, B).transpose(1, 0, 2)
        ).astype(FP8)                                     # [128, CH, B]
        unm = us.transpose(1, 0, 2).reshape(M, 512)       # [n, l*64+r]
        up4 = unm.reshape(CP, 2, 128, 512).transpose(2, 0, 1, 3)
        upa = np.ascontiguousarray(up4[..., 0:192]).astype(FP8)
        upb = np.ascontiguousarray(up4[..., 192:512]).astype(FP8)
        ut = np.ascontiguousarray(
            us.transpose(0, 2, 1).reshape(512, M).reshape(4, 128, M)
        ).astype(FP8)
        msk = np.zeros((64, 8, 192), dtype=BF16)
        for k in range(8):
            if (k // 4) == ((c // 4) ^ 1):
                msk[:, k, 0:64] = 1.0     # level 0
            if (k // 2) == ((c // 2) ^ 1):
                msk[:, k, 64:128] = 1.0   # level 1
            if k == c ^ 1:
                msk[:, k, 128:192] = 1.0  # level 2
        in_maps.append(
            {"xt": xt, "upa": upa, "upb": upb, "ut": ut, "mask": msk,
             "idn": idn}
        )
    return in_maps


last_results = None


def kernel(x, diag, u):
    global last_results
    from concourse.bass_utils import run_bass_kernel_spmd

    if "nc" not in _cached:
        _cached["nc"] = _build_bass()
    nc = _cached["nc"]

    in_maps = _pack_inputs(x, diag, u)
    res = run_bass_kernel_spmd(nc, in_maps, core_ids=list(range(NCORES)))
    last_results = res

    x2 = np.asarray(x, dtype=np.float32).reshape(B, N)
    d2 = np.asarray(diag, dtype=np.float32).reshape(1, N)
    y = d2 * x2
    inv = 1.0 / (USCALE * USCALE)
    for c in range(NCORES):
        y[:, c * M : (c + 1) * M] += res.results[c]["corr"].astype(np.float32) * inv
    return y.reshape(B, N, 1).astype(np.float32)


# revision 30
# speedup vs baseline: 1.3771x; 1.3771x over previous
"""HODLR matvec kernel for 8 TRN2 NeuronCores (Bass/Tile), v3.

Sharding: node axis split into 8 contiguous slices of 32768 nodes.
DMA-bound design (~40MB/core at ~360GB/s): compute hides under the
input stream.

Per core:
  stream order: xt | UPA (u levels 0-2, n-major) | UPB (levels 3-7) |
    UT23 (u^T levels 4-7, fully prefetched) | UT01 (levels 0-3, ring),
    with corr output DMAs at the queue tails.
  projection  one DR fp8 matmul per chunk-pair per level-group with x
    as the stationary; per-block PSUM accumulators (L7 ping-pongs two
    banks, coarser levels ride 1-bank rings; levels 0-2 close as soon
    as UPA lands, launching the AllGather early).
  collective  AllGather of the [64, 192] level-0..2 partials; masked
    receive-combine (sender-invariant).
  transposes  PE is_transpose matmuls turn sibling-selected t^T slots
    into fp8 DR stationaries (sibling XOR encoded in the source APs).
  expansion   per 512-node group: one DR matmul vs UT23 (levels 4-7,
    PSUM start) and one vs UT01 (levels 0-3, stop); drains write bf16
    corr staged per 4096 block.
u/x fed as fp8e4m3 (u scaled by USCALE; host divides the correction by
USCALE^2 and adds diag*x in fp32).
"""

import os
import sys

sys.path.insert(0, "/opt/trn_rl_repo")

import numpy as np
import ml_dtypes

BF16 = ml_dtypes.bfloat16
FP8 = ml_dtypes.float8_e4m3

B = 64
N = 262144
NCORES = 8
M = N // NCORES          # 32768 nodes per core
R = 64
DEPTH = 8
CH = M // 128            # 256 chunks of 128 nodes
CP = CH // 2             # 128 chunk-pairs (256 nodes, DR k-tiles)
USCALE = 64.0

_cached = {}


def _build_bass():
    import concourse.bacc as bacc
    import concourse.tile as tile
    import concourse.mybir as mybir
    from contextlib import ExitStack

    BF = mybir.dt.bfloat16
    F8 = mybir.dt.float8e4
    F32 = mybir.dt.float32
    ADD = mybir.AluOpType.add
    MULT = mybir.AluOpType.mult
    DR = mybir.MatmulPerfMode.DoubleRow

    nc = bacc.Bacc(
        "TRN2",
        target_bir_lowering=False,
        debug=False,
        enable_asserts=False,
        num_devices=NCORES,
    )

    xt_d = nc.dram_tensor("xt", [128, CH, B], F8, kind="ExternalInput").ap()
    upa_d = nc.dram_tensor("upa", [128, CP, 2, 192], F8, kind="ExternalInput").ap()
    upb_d = nc.dram_tensor("upb", [128, CP, 2, 320], F8, kind="ExternalInput").ap()
    ut_d = nc.dram_tensor("ut", [4, 128, M], F8, kind="ExternalInput").ap()
    msk_d = nc.dram_tensor("mask", [64, 8, 192], BF, kind="ExternalInput").ap()
    idn_d = nc.dram_tensor("idn", [64, 64], BF, kind="ExternalInput").ap()
    corr_d = nc.dram_tensor("corr", [B, M], BF, kind="ExternalOutput").ap()

    NSL = 8            # slices for UPA/UPB streams
    SLC = CP // NSL    # 16 chunk-pairs per slice
    # TT slot index layout (t^T vectors, [B, 64] each)
    SL7 = 0            # 32 slots: level-7 blocks
    SL6 = 32           # 16
    SL5 = 48           # 8
    SL4 = 56           # 4
    SL3 = 60           # 2
    SL0 = 62           # 3: combined levels 0,1,2 (post-collective)
    NSLOT = 65

    with tile.TileContext(nc) as tc, ExitStack() as ctx:
        const = ctx.enter_context(tc.tile_pool(name="const", bufs=1))
        upap = ctx.enter_context(tc.tile_pool(name="upap", bufs=3))
        upbp = ctx.enter_context(tc.tile_pool(name="upbp", bufs=3))
        utap = ctx.enter_context(tc.tile_pool(name="utap", bufs=8))
        utbp = ctx.enter_context(tc.tile_pool(name="utbp", bufs=4))
        ttp = ctx.enter_context(tc.tile_pool(name="ttp", bufs=1))
        statp = ctx.enter_context(tc.tile_pool(name="statp", bufs=1))
        yop = ctx.enter_context(tc.tile_pool(name="yop", bufs=3))
        # PSUM: 8 banks: prA bufs=2 (L7 ping-pong, then transposes),
        # prB..prF bufs=1 (L6..L3 + psF, then the expansion eps ring
        # together with prA).
        prA = ctx.enter_context(tc.tile_pool(name="prA", bufs=2, space="PSUM"))
        prB = ctx.enter_context(tc.tile_pool(name="prB", bufs=1, space="PSUM"))
        prC = ctx.enter_context(tc.tile_pool(name="prC", bufs=1, space="PSUM"))
        prD = ctx.enter_context(tc.tile_pool(name="prD", bufs=1, space="PSUM"))
        prE = ctx.enter_context(tc.tile_pool(name="prE", bufs=1, space="PSUM"))
        prF = ctx.enter_context(tc.tile_pool(name="prF", bufs=1, space="PSUM"))
        dram = ctx.enter_context(tc.tile_pool(name="dram", bufs=1, space="DRAM"))

        def drainer():
            # gpsimd cannot access PSUM; DVE mostly, scheduler-picked 1/3
            i = 0
            while True:
                yield (nc.vector if i % 3 != 2 else nc.any)
                i += 1

        dr_eng = drainer()

        # ---------------- constants ----------------
        xt = const.tile([128, CH, B], F8, tag="xt")
        nc.sync.dma_start(xt[:, 0 : CH // 2, :], xt_d[:, 0 : CH // 2, :])
        nc.scalar.dma_start(xt[:, CH // 2 :, :], xt_d[:, CH // 2 :, :])
        msk = const.tile([64, 8, 192], BF, tag="msk")
        nc.scalar.dma_start(msk[:], msk_d[:])
        idn = const.tile([64, 64], BF, tag="idn")
        nc.scalar.dma_start(idn[:], idn_d[:])

        TT = ttp.tile([64, NSLOT, 64], BF, tag="TT")

        # ---------------- projection: levels 0-2 (UPA) ----------------
        psF_t = prF.tile([64, 512], F32, tag="t", name="psF")
        psF = psF_t[:, 0:192]
        for s in range(NSL):
            ua = upap.tile([128, SLC, 2, 192], F8, tag="upa", name=f"upa{s}")
            (nc.sync if s % 2 == 0 else nc.scalar).dma_start(
                ua[:], upa_d[:, SLC * s : SLC * (s + 1), :, :]
            )
            for i in range(SLC):
                cp = SLC * s + i
                nc.tensor.matmul(
                    psF,
                    xt[:, 2 * cp : 2 * cp + 2, :],
                    ua[:, i, :, :],
                    start=(cp == 0),
                    stop=(cp == CP - 1),
                    perf_mode=DR,
                )

        # collective staging + AllGather (launches ~25% into the stream)
        b_in = dram.tile([64, 192], BF, tag="b_in")
        b_out = dram.tile([8, 64, 192], BF, tag="b_out", addr_space="Shared")
        sb_in = statp.tile([64, 192], BF, tag="sb_in")
        nc.vector.tensor_copy(sb_in[:], psF)
        nc.gpsimd.dma_start(b_in[:], sb_in[:])
        nc.gpsimd.collective_compute(
            "AllGather",
            mybir.AluOpType.bypass,
            replica_groups=[list(range(NCORES))],
            ins=[b_in.opt()],
            outs=[b_out.opt()],
        )
        recv = statp.tile([64, 8, 192], BF, tag="recv")
        for k in range(8):
            nc.gpsimd.dma_start(recv[:, k, :], b_out[k, :, :])
        # masked receive-combine -> TT slots 62..64 (levels 0,1,2)
        mtmp = statp.tile([64, 8, 192], BF, tag="mtmp")
        for k in range(8):
            nc.vector.tensor_tensor(
                mtmp[:, k, :], recv[:, k, :], msk[:, k, :], op=MULT
            )
        acc01 = statp.tile([64, 192], BF, tag="acc01")
        nc.vector.tensor_tensor(acc01[:], mtmp[:, 0, :], mtmp[:, 1, :], op=ADD)
        for k in range(2, 7):
            nc.vector.tensor_tensor(acc01[:], acc01[:], mtmp[:, k, :], op=ADD)
        nc.vector.tensor_tensor(
            TT[:, SL0 : SL0 + 3, :], acc01[:], mtmp[:, 7, :], op=ADD
        )

        # ---------------- projection: levels 3-7 (UPB) ----------------
        # per-level per-block psum accumulators; close cadences (cps):
        # L3:64 L4:32 L5:16 L6:8 L7:4
        lvl_pool = {3: prE, 4: prD, 5: prC, 6: prB, 7: prA}
        lvl_slot = {3: SL3, 4: SL4, 5: SL5, 6: SL6, 7: SL7}
        lvl_cad = {3: 64, 4: 32, 5: 16, 6: 8, 7: 4}
        cur = {}
        for s in range(NSL):
            ub = upbp.tile([128, SLC, 2, 320], F8, tag="upb", name=f"upb{s}")
            (nc.sync if s % 2 == 0 else nc.scalar).dma_start(
                ub[:], upb_d[:, SLC * s : SLC * (s + 1), :, :]
            )
            for i in range(SLC):
                cp = SLC * s + i
                for l in (7, 6, 5, 4, 3):
                    cad = lvl_cad[l]
                    blk = cp // cad
                    j = l - 3
                    if cp % cad == 0:
                        cur[l] = lvl_pool[l].tile(
                            [64, 512], F32, tag="t", name=f"ps{l}_{blk}"
                        )
                    nc.tensor.matmul(
                        cur[l][:, 0:64],
                        xt[:, 2 * cp : 2 * cp + 2, :],
                        ub[:, i, :, 64 * j : 64 * j + 64],
                        start=(cp % cad == 0),
                        stop=(cp % cad == cad - 1),
                        perf_mode=DR,
                    )
                    if cp % cad == cad - 1:
                        next(dr_eng).tensor_copy(
                            TT[:, lvl_slot[l] + blk, :], cur[l][:, 0:64]
                        )

        # ---------------- sibling transposes -> fp8 stationaries -------
        # P23[:, m7, 0, :] = (t4sib | t5sib); [:, m7, 1, :] = (t6sib | t7sib)
        # P01[:, m3, 0, :] = (L0comb | L1comb); [:, m3, 1, :] = (L2comb | t3sib)
        P23 = statp.tile([128, 32, 2, B], F8, tag="P23")
        P01 = statp.tile([128, 2, 2, B], F8, tag="P01")

        def transpose_pair(dst_ap, slot_top, slot_bot, nm):
            # full-bank scratch keeps start-flag zeroing away from
            # neighbors
            tp = prA.tile([128, 1024], BF, tag="t", name=nm)
            nc.tensor.matmul(
                tp[0:64, 0:B], TT[:, slot_top, :], idn[:],
                is_transpose=True,
            )
            nc.tensor.matmul(
                tp[64:128, 0:B], TT[:, slot_bot, :], idn[:],
                is_transpose=True,
            )
            next(dr_eng).tensor_copy(dst_ap, tp[:, 0:B])

        for m7 in range(32):
            transpose_pair(
                P23[:, m7, 0, :],
                SL4 + ((m7 // 8) ^ 1),
                SL5 + ((m7 // 4) ^ 1),
                f"tp45_{m7}",
            )
            transpose_pair(
                P23[:, m7, 1, :],
                SL6 + ((m7 // 2) ^ 1),
                SL7 + (m7 ^ 1),
                f"tp67_{m7}",
            )
        # P01 last: it waits on the collective, and the PE runs in order
        for m3 in range(2):
            transpose_pair(P01[:, m3, 0, :], SL0 + 0, SL0 + 1, f"tp01_{m3}")
            transpose_pair(
                P01[:, m3, 1, :], SL0 + 2, SL3 + (m3 ^ 1), f"tp23_{m3}"
            )

        # ---------------- UT prefetch ----------------
        # UT23 fully resident (no backpressure); UT01 4-deep ring
        u23s = []
        for gb in range(8):
            u23 = utap.tile([128, 2, 4096], F8, tag="ut23", name=f"u23_{gb}")
            for f in range(2):
                nc.sync.dma_start(
                    u23[:, f, :], ut_d[2 + f, :, 4096 * gb : 4096 * (gb + 1)]
                )
            u23s.append(u23)

        # ---------------- expansion (per 4096-node block) ----------------
        erng = (prB, prC, prD, prE, prF, prA)
        for gb in range(8):
            u01 = utbp.tile([128, 2, 4096], F8, tag="ut01", name=f"u01_{gb}")
            for f in range(2):
                nc.scalar.dma_start(
                    u01[:, f, :], ut_d[f, :, 4096 * gb : 4096 * (gb + 1)]
                )
            yo = yop.tile([B, 4096], BF, tag="yo", name=f"yo{gb}")
            for gg in range(8):
                g = 8 * gb + gg
                eps_t = erng[g % 6].tile(
                    [64, 512], F32, tag="t", name=f"eps{g}"
                )
                eps = eps_t[:, 0:512]
                sl = slice(512 * gg, 512 * (gg + 1))
                nc.tensor.matmul(
                    eps, P23[:, g // 2, :, :], u23s[gb][:, :, sl],
                    start=True, stop=False, perf_mode=DR,
                )
                nc.tensor.matmul(
                    eps, P01[:, g // 32, :, :], u01[:, :, sl],
                    start=False, stop=True, perf_mode=DR,
                )
                next(dr_eng).tensor_copy(yo[:, sl], eps)
            (nc.sync if gb % 2 == 0 else nc.scalar).dma_start(
                corr_d[:, 4096 * gb : 4096 * (gb + 1)], yo[:]
            )

    nc.compile()
    return nc


def _pack_inputs(x, diag, u):
    """Build per-core input maps. x (B,N,1) f32, u (DEPTH,N,R) f32."""
    in_maps = []
    x2 = np.asarray(x).reshape(B, N)
    u3 = np.asarray(u)
    idn = np.eye(64, dtype=BF16)
    for c in range(NCORES):
        base = c * M
        xsl = x2[:, base : base + M]                      # (B, M)
        us = (u3[:, base : base + M, :] * USCALE).astype(np.float32)
        xt = np.ascontiguousarray(
            xsl.T.reshape(CH, 128, B).transpose(1, 0, 2)
        ).astype(FP8)                                     # [128, CH, B]
        unm = us.transpose(1, 0, 2).reshape(M, 512)       # [n, l*64+r]
        up4 = unm.reshape(CP, 2, 128, 512).transpose(2, 0, 1, 3)
        upa = np.ascontiguousarray(up4[..., 0:192]).astype(FP8)
        upb = np.ascontiguousarray(up4[..., 192:512]).astype(FP8)
        ut = np.ascontiguousarray(
            us.transpose(0, 2, 1).reshape(512, M).reshape(4, 128, M)
        ).astype(FP8)
        msk = np.zeros((64, 8, 192), dtype=BF16)
        for k in range(8):
            if (k // 4) == ((c // 4) ^ 1):
                msk[:, k, 0:64] = 1.0     # level 0
            if (k // 2) == ((c // 2) ^ 1):
                msk[:, k, 64:128] = 1.0   # level 1
            if k == c ^ 1:
                msk[:, k, 128:192] = 1.0  # level 2
        in_maps.append(
            {"xt": xt, "upa": upa, "upb": upb, "ut": ut, "mask": msk,
             "idn": idn}
        )
    return in_maps


last_results = None


def kernel(x, diag, u):
    global last_results
    from concourse.bass_utils import run_bass_kernel_spmd

    if "nc" not in _cached:
        _cached["nc"] = _build_bass()
    nc = _cached["nc"]

    in_maps = _pack_inputs(x, diag, u)
    res = run_bass_kernel_spmd(nc, in_maps, core_ids=list(range(NCORES)))
    last_results = res

    x2 = np.asarray(x, dtype=np.float32).reshape(B, N)
    d2 = np.asarray(diag, dtype=np.float32).reshape(1, N)
    y = d2 * x2
    inv = 1.0 / (USCALE * USCALE)
    for c in range(NCORES):
        y[:, c * M : (c + 1) * M] += res.results[c]["corr"].astype(np.float32) * inv
    return y.reshape(B, N, 1).astype(np.float32)
